# revision 26
# baseline (speedup 1.0000x reference)
"""GCN layer kernel for 8 Trainium2 NeuronCores.

Computes: out = relu(dinv[:,None] * ((adj+I).T @ (dinv[:,None] * (x@W))) + b)
where dinv = rsqrt(colsum(adj) + 1).

Strategy: shard adj by COLUMNS across the 8 cores. Column block c (together
with the full scaled source features z) is exactly what's needed to produce
output rows [c*2048, (c+1)*2048) -- so no device collectives are needed.
Host preprocessing folds the self-loop (+I) into the adjacency block and
casts it to fp8e4m3 ({0,1,2} exact) -- HBM traffic on the 1 GB adjacency is
the roofline. Each core streams its block once.

Mode "dr" (default): DoubleRow fp8 matmuls. The stationary operand packs
[z8 | r8] (fp8 main + x16-scaled fp8 residual of z*512) into 128 columns;
each matmul contracts a PAIR of 128-row k-tiles (256-deep DoubleRow) at
2 MACs/cell/cycle -- 2x the bf16-pair PE rate. PSUM rows 0-63 accumulate
A^T z8, rows 64-127 A^T r8; a tiny bf16 fold matmul (weights 2^-9 / 2^-13)
recombines them, then dinv * + bias + relu + bf16 store.

Mode "fp8pair": previous baseline -- bf16 z stationary, two col-tiled
64-wide matmuls run concurrently (full 128x128 array, 1 MAC/cell/cycle).
"""

import sys

import numpy as np

if "/opt/trn_rl_repo" not in sys.path:
    sys.path.insert(0, "/opt/trn_rl_repo")

import ml_dtypes

N = 16384
F = 64
NCORES = 8
NB = N // NCORES  # 2048 columns (= output rows) per core
P = 128
KT = N // P  # 128 k-tiles of 128 source rows each
MM_N = 512  # moving-operand free dim per matmul (one PSUM bank of f32)
DMA_BATCH = 4  # k-tiles per dma_start
APOOL_BUFS = 6  # in-flight A-tile slots (prefetch depth)
ALT_DMA = True  # alternate A-tile DMAs between the SP and ACT HWDGE rings
SZ = 512.0  # z pre-scale for fp8 (power of 2)
SR = 16.0  # residual scale (power of 2)
MODE = "dr"  # variant kernel() uses

_BASS_CACHE: dict = {}


def _mode_flags(mode):
    fp8 = mode in ("fp8", "fp8pair", "dma8", "mm8", "mmpair")
    pair = mode in ("pair", "fp8pair", "mmpair")
    dr = mode in ("dr", "mmdr", "mmdr1", "dmadr", "dr2", "mmdr2", "dmadr2")
    dr2 = mode in ("dr2", "mmdr2", "dmadr2")
    resident = mode in ("mm", "mm8", "mmpair", "mmdr", "mmdr1", "mmdr2")
    dmaonly = mode in ("dma", "dma8", "dmadr", "dmadr2")
    return fp8, pair, dr, dr2, resident, dmaonly


def _build_bass(reps: int = 1, mode: str = "full"):
    """Build the per-core Bass module. reps>1 repeats the whole compute
    (same inputs/outputs) inside one NEFF -- used only for benchmarking
    device time independent of dispatch overhead. Probe modes: "dma*"
    (loads only, no matmul), "mm*" (matmuls from a single resident tile,
    1/64th of the DMA traffic)."""
    key = (reps, mode, DMA_BATCH, APOOL_BUFS, ALT_DMA)
    if key in _BASS_CACHE:
        return _BASS_CACHE[key]

    import concourse.mybir as mybir
    import concourse.tile as tile
    from concourse import bacc

    nc = bacc.Bacc("TRN2", target_bir_lowering=False, debug=False,
                   num_devices=NCORES)

    fp8, pair, dr, dr2, resident, dmaonly = _mode_flags(mode)
    a_dt = mybir.dt.float8e4 if (fp8 or dr) else mybir.dt.bfloat16
    # pair mode: two col-group-tiled matmuls run concurrently. Each 512-chunk
    # nn gets its own PSUM bank (columns nn*512) with even chunks on
    # partitions 0-63 and odd chunks on 64-127, so no two accumulation
    # groups share a bank.
    b_p = 2 * F if pair else F       # bias/dinv partition count
    a_in = nc.dram_tensor("a", [N, NB], a_dt, kind="ExternalInput")
    if dr:
        z_in = nc.dram_tensor("z", [P, KT * P], mybir.dt.float8e4,
                              kind="ExternalInput")
        d_in = nc.dram_tensor("dinv", [F, NB], mybir.dt.bfloat16,
                              kind="ExternalInput")
        f_in = nc.dram_tensor("fold", [P, F], mybir.dt.bfloat16,
                              kind="ExternalInput")
        b_in = nc.dram_tensor("bvec", [F, 1], mybir.dt.float32,
                              kind="ExternalInput")
        o_out = nc.dram_tensor("o", [F, NB], mybir.dt.bfloat16,
                               kind="ExternalOutput")
    else:
        z_in = nc.dram_tensor("z", [P, KT * F], mybir.dt.bfloat16,
                              kind="ExternalInput")
        b_in = nc.dram_tensor("bvec", [b_p, 1], mybir.dt.float32,
                              kind="ExternalInput")
        if fp8:
            d_in = nc.dram_tensor("dinv", [b_p, NB], mybir.dt.float32,
                                  kind="ExternalInput")
        o_out = nc.dram_tensor("o", [F, NB], mybir.dt.float32,
                               kind="ExternalOutput")

    kb = DMA_BATCH
    assert not dr or kb % 2 == 0
    # [KT/kb, 128, kb, NB]: group kb consecutive k-tiles into one DMA
    a_tiles = a_in.ap().rearrange("(g t p) i -> g p t i", t=kb, p=P)

    relu = mybir.ActivationFunctionType.Relu
    NCH = NB // MM_N  # 512-wide output chunks

    with tile.TileContext(nc) as tc:
        with (
            tc.tile_pool(name="singles", bufs=1) as singles,
            tc.tile_pool(name="apool", bufs=APOOL_BUFS) as apool,
            tc.tile_pool(name="psum", bufs=1, space="PSUM") as psum_pool,
            tc.tile_pool(name="psum2", bufs=2, space="PSUM") as psum2_pool,
        ):
            if dr2:
                # z: [128, KT tiles, 2 members (z8|r8), 64 features]
                z_sb = singles.tile([P, KT, 2, F], mybir.dt.float8e4)
                nc.sync.dma_start(
                    z_sb[:], z_in.ap().rearrange("p (kt i f) -> p kt i f",
                                                 i=2, f=F))
            elif dr:
                # z pairs: [128, KT/2 pairs, 2 members, 128 cols (z8|r8)]
                z_sb = singles.tile([P, KT // 2, 2, P], mybir.dt.float8e4)
                nc.sync.dma_start(
                    z_sb[:], z_in.ap().rearrange("p (j i m) -> p j i m",
                                                 i=2, m=P))
            else:
                z_sb = singles.tile([P, KT * F], mybir.dt.bfloat16)
                nc.sync.dma_start(z_sb[:], z_in.ap())
                if fp8:
                    d_sb = singles.tile([b_p, NB], mybir.dt.float32,
                                        tag="d_sb")
                    nc.sync.dma_start(d_sb[:], d_in.ap())
            if dr:
                d_sb = singles.tile([F, NB], mybir.dt.bfloat16, tag="d_sb")
                nc.sync.dma_start(d_sb[:], d_in.ap())
                f_sb = singles.tile([P, F], mybir.dt.bfloat16, tag="f_sb")
                nc.sync.dma_start(f_sb[:], f_in.ap())
            b_sb = singles.tile([F if dr else b_p, 1], mybir.dt.float32)
            nc.sync.dma_start(b_sb[:], b_in.ap())

            mm_tile = None
            if resident:
                mm_tile = singles.tile([P, kb, NB], a_dt, tag="mm_tile")
                nc.sync.dma_start(mm_tile[:], a_tiles[0])

            for _rep in range(reps):
                if dr:
                    # one PSUM bank per 512-col chunk so each chunk's
                    # accumulator releases as soon as its epilogue copy is
                    # done -- the next pass's chunk-nn matmuls only wait on
                    # chunk-nn, not the whole epilogue.
                    pss = [psum_pool.tile([P, MM_N], mybir.dt.float32,
                                          name=f"psc{nn}")
                           for nn in range(NCH)]
                    perf = mybir.MatmulPerfMode.DoubleRow
                    for g in range(KT // kb):
                        if resident:
                            at = mm_tile
                        else:
                            at = apool.tile([P, kb, NB], a_dt)
                            eng = nc.scalar if (ALT_DMA and g % 2) else nc.sync
                            eng.dma_start(at[:], a_tiles[g])
                        if dmaonly:
                            continue
                        if dr2:
                            # two concurrent 64-col DoubleRow chains: even
                            # k-tiles on col-group 0 (psum rows 0-63), odd on
                            # col-group 1 (rows 64-127). Each matmul pairs
                            # (z8, r8) against the SAME A tile (stride-0
                            # broadcast) so the residual add is free.
                            for t in range(kb):
                                kt = g * kb + t
                                h = kt % 2
                                zkt = z_sb[:, kt, :, :]
                                for nn in range(NCH):
                                    sf = slice(nn * MM_N, (nn + 1) * MM_N)
                                    nc.tensor.matmul(
                                        pss[nn][h * F:(h + 1) * F, :],
                                        lhsT=zkt,
                                        rhs=at[:, t:t + 1, sf]
                                        .broadcast_to([P, 2, MM_N]),
                                        start=(kt <= 1),
                                        stop=(kt >= KT - 2),
                                        perf_mode=perf,
                                        tile_position=(0, h * F),
                                    )
                            continue
                        for jj in range(kb // 2):
                            j = g * (kb // 2) + jj
                            zpair = z_sb[:, 0 if mode == "mmdr1"
                                         else j % (KT // 2), :, :]
                            for nn in range(NCH):
                                nc.tensor.matmul(
                                    pss[nn][:],
                                    lhsT=zpair,
                                    rhs=at[:, 2 * jj:2 * jj + 2,
                                           nn * MM_N:(nn + 1) * MM_N],
                                    start=(j == 0),
                                    stop=(j == KT // 2 - 1),
                                    perf_mode=perf,
                                )

                    s_sb = singles.tile([P, NB], mybir.dt.bfloat16,
                                        tag="s_sb")
                    o1_sb = singles.tile([F, NB], mybir.dt.bfloat16,
                                         tag="o1_sb")
                    out_sb = singles.tile([F, NB], mybir.dt.bfloat16,
                                          tag="out_sb")
                    if dmaonly:
                        nc.vector.tensor_copy(out_sb[:, :F],
                                              z_sb[:F, 0, 0, :F])
                        nc.sync.dma_start(o_out.ap(), out_sb[:])
                    else:
                        for nn in range(NCH):
                            sf = slice(nn * MM_N, (nn + 1) * MM_N)
                            nc.vector.tensor_copy(s_sb[:, sf], pss[nn][:])
                            ps2 = psum2_pool.tile([F, MM_N], mybir.dt.float32)
                            nc.tensor.matmul(ps2[:], lhsT=f_sb[:],
                                             rhs=s_sb[:, sf],
                                             start=True, stop=True)
                            nc.vector.tensor_mul(o1_sb[:, sf], ps2[:],
                                                 d_sb[:, sf])
                            nc.scalar.activation(out_sb[:, sf], o1_sb[:, sf],
                                                 relu, bias=b_sb[:],
                                                 scale=1.0)
                            nc.sync.dma_start(o_out.ap()[:, sf],
                                              out_sb[:, sf])
                    continue

                ps = psum_pool.tile([b_p, NB], mybir.dt.float32)

                for g in range(KT // kb):
                    if resident:
                        at = mm_tile
                    else:
                        at = apool.tile([P, kb, NB], a_dt)
                        eng = nc.scalar if (ALT_DMA and g % 2) else nc.sync
                        eng.dma_start(at[:], a_tiles[g])
                    if dmaonly:
                        continue
                    for t in range(kb):
                        kt = g * kb + t
                        zk = z_sb[:, (kt % KT) * F:((kt % KT) + 1) * F]
                        if pair:
                            for nn in range(NCH):
                                h = nn % 2
                                nc.tensor.matmul(
                                    ps[h * F:(h + 1) * F,
                                       nn * MM_N:(nn + 1) * MM_N],
                                    lhsT=zk,
                                    rhs=at[:, t, nn * MM_N:(nn + 1) * MM_N],
                                    start=(kt == 0),
                                    stop=(kt == KT - 1),
                                    tile_position=(0, h * F),
                                )
                        else:
                            for nn in range(NCH):
                                nc.tensor.matmul(
                                    ps[:, nn * MM_N:(nn + 1) * MM_N],
                                    lhsT=zk,
                                    rhs=at[:, t, nn * MM_N:(nn + 1) * MM_N],
                                    start=(kt == 0),
                                    stop=(kt == KT - 1),
                                )

                out_sb = singles.tile([b_p, NB], mybir.dt.float32,
                                      tag="out_sb")
                if dmaonly:
                    nc.vector.tensor_copy(out_sb[:F, :F], z_sb[:F, :F])
                    nc.sync.dma_start(o_out.ap(), out_sb[:F, :])
                elif pair:
                    # touch only the written PSUM quadrants
                    for nn in range(NCH):
                        h = nn % 2
                        sp = slice(h * F, (h + 1) * F)
                        sf = slice(nn * MM_N, (nn + 1) * MM_N)
                        if fp8:
                            nc.vector.tensor_mul(out_sb[sp, sf], ps[sp, sf],
                                                 d_sb[sp, sf])
                            nc.scalar.activation(out_sb[sp, sf],
                                                 out_sb[sp, sf], relu,
                                                 bias=b_sb[sp], scale=1.0)
                        else:
                            nc.scalar.activation(out_sb[sp, sf], ps[sp, sf],
                                                 relu, bias=b_sb[sp],
                                                 scale=1.0)
                        nc.sync.dma_start(o_out.ap()[:, sf], out_sb[sp, sf])
                elif fp8:
                    nc.vector.tensor_mul(out_sb[:], ps[:], d_sb[:])
                    nc.scalar.activation(out_sb[:], out_sb[:], relu,
                                         bias=b_sb[:], scale=1.0)
                    nc.sync.dma_start(o_out.ap(), out_sb[:])
                else:
                    nc.scalar.activation(out_sb[:], ps[:], relu,
                                         bias=b_sb[:], scale=1.0)
                    nc.sync.dma_start(o_out.ap(), out_sb[:])

    nc.compile()
    _BASS_CACHE[key] = nc
    return nc


def _host_prep(x, adj, W, b, mode=None):
    """Host-side sharding/preprocessing -> per-core input maps."""
    if mode is None:
        mode = MODE
    fp8, pair, dr, dr2, resident, dmaonly = _mode_flags(mode)
    x = np.asarray(x, dtype=np.float32)
    adj = np.asarray(adj, dtype=np.float32)
    W = np.asarray(W, dtype=np.float32)
    b = np.asarray(b, dtype=np.float32)

    deg = adj.sum(axis=0) + 1.0
    dinv = np.where(deg > 0, 1.0 / np.sqrt(deg), 0.0).astype(np.float32)

    z = (dinv[:, None] * (x @ W)).astype(np.float32)  # [N, F]

    idx = np.arange(NB)
    in_maps = []

    if dr:
        zq = z * SZ
        z8 = zq.astype(ml_dtypes.float8_e4m3)
        fold = np.zeros((P, F), np.float32)
        fold[np.arange(F), np.arange(F)] = 1.0 / SZ
        if dr2:
            r = zq - z8.astype(np.float32)
            r8 = r.astype(ml_dtypes.float8_e4m3)  # unscaled residual
            # z_dev[p, kt*128 + i*64 + f] = (z8 if i==0 else r8)[kt*128+p, f]
            zr = np.stack([z8.reshape(KT, P, F), r8.reshape(KT, P, F)],
                          axis=2)  # [KT, P, 2, F]
            z_dev = np.ascontiguousarray(
                zr.transpose(1, 0, 2, 3).reshape(P, KT * P))
            fold[F + np.arange(F), np.arange(F)] = 1.0 / SZ
        else:
            r = (zq - z8.astype(np.float32)) * SR
            r8 = r.astype(ml_dtypes.float8_e4m3)
            zr = np.concatenate([z8, r8], axis=1)  # [N, 128] fp8
            # z_dev[p, j*256 + i*128 + m] = zr[(2j+i)*128 + p, m]
            z_dev = np.ascontiguousarray(
                zr.reshape(KT // 2, 2, P, P).transpose(2, 0, 1, 3)
                .reshape(P, KT * P))
            fold[F + np.arange(F), np.arange(F)] = 1.0 / (SZ * SR)
        fold = fold.astype(ml_dtypes.bfloat16)
        b_dev = np.ascontiguousarray(b.reshape(F, 1))
        for c in range(NCORES):
            cs = c * NB
            blk = adj[:, cs:cs + NB].copy()
            blk[cs + idx, idx] += 1.0  # self-loop (+I)
            d64 = np.ascontiguousarray(
                np.broadcast_to(dinv[cs:cs + NB], (F, NB))
            ).astype(ml_dtypes.bfloat16)
            in_maps.append({
                "a": blk.astype(ml_dtypes.float8_e4m3),
                "z": z_dev,
                "dinv": d64,
                "fold": fold,
                "bvec": b_dev,
            })
        return in_maps

    # k-major layout: z_sb[p, kt*F + f] = z[kt*128 + p, f]
    z_dev = np.ascontiguousarray(
        z.reshape(KT, P, F).transpose(1, 0, 2).reshape(P, KT * F)
    ).astype(ml_dtypes.bfloat16)

    if pair:
        b_dev = np.ascontiguousarray(
            np.concatenate([b, b]).reshape(2 * F, 1))
    else:
        b_dev = np.ascontiguousarray(b.reshape(F, 1))

    def _pair_dinv(dc):
        # [128, NB]: chunk nn lives at [64*(nn%2):64*(nn%2+1), nn*512:...]
        d = np.zeros((2 * F, NB), np.float32)
        for nn in range(NB // MM_N):
            h = nn % 2
            d[h * F:(h + 1) * F, nn * MM_N:(nn + 1) * MM_N] = \
                dc[nn * MM_N:(nn + 1) * MM_N]
        return d

    for c in range(NCORES):
        cs = c * NB
        if fp8:
            # adjacency stays exact {0,1,2} in fp8; dinv applied on device
            blk = adj[:, cs:cs + NB].copy()
            blk[cs + idx, idx] += 1.0  # self-loop (+I)
            dc = dinv[cs:cs + NB]
            m = {
                "a": blk.astype(ml_dtypes.float8_e4m3),
                "z": z_dev,
                "bvec": b_dev,
                "dinv": (_pair_dinv(dc) if pair else np.ascontiguousarray(
                    np.broadcast_to(dc, (F, NB)))),
            }
        else:
            blk = adj[:, cs:cs + NB] * dinv[cs:cs + NB][None, :]
            blk[cs + idx, idx] += dinv[cs + idx]  # fold self-loop (+I)
            m = {
                "a": blk.astype(ml_dtypes.bfloat16),
                "z": z_dev,
                "bvec": b_dev,
            }
        in_maps.append(m)
    return in_maps


def _assemble(results, mode=None):
    """Device outputs -> full [N, F] output."""
    if mode is None:
        mode = MODE
    out = np.empty((N, F), dtype=np.float32)
    for c in range(NCORES):
        o = results[c]["o"]
        out[c * NB:(c + 1) * NB, :] = o.astype(np.float32).T
    return out


def kernel(x, adj, W, b):
    from concourse import bass_utils

    nc = _build_bass(mode=MODE)
    in_maps = _host_prep(x, adj, W, b, mode=MODE)
    res = bass_utils.run_bass_kernel_spmd(nc, in_maps,
                                          core_ids=list(range(NCORES)))
    return _assemble(res.results, mode=MODE)


# revision 27
# speedup vs baseline: 1.0539x; 1.0539x over previous
"""GCN layer kernel for 8 Trainium2 NeuronCores.

Computes: out = relu(dinv[:,None] * ((adj+I).T @ (dinv[:,None] * (x@W))) + b)
where dinv = rsqrt(colsum(adj) + 1).

Strategy: shard adj by COLUMNS across the 8 cores. Column block c (together
with the full scaled source features z) is exactly what's needed to produce
output rows [c*2048, (c+1)*2048) -- so no device collectives are needed.
Host preprocessing folds the self-loop (+I) into the adjacency block and
casts it to fp8e4m3 ({0,1,2} exact) -- HBM traffic on the 1 GB adjacency is
the roofline. Each core streams its block once.

Mode "dr" (default): DoubleRow fp8 matmuls. The stationary operand packs
[z8 | r8] (fp8 main + x16-scaled fp8 residual of z*512) into 128 columns;
each matmul contracts a PAIR of 128-row k-tiles (256-deep DoubleRow) at
2 MACs/cell/cycle -- 2x the bf16-pair PE rate. PSUM rows 0-63 accumulate
A^T z8, rows 64-127 A^T r8; a tiny bf16 fold matmul (weights 2^-9 / 2^-13)
recombines them, then dinv * + bias + relu + bf16 store.

Mode "fp8pair": previous baseline -- bf16 z stationary, two col-tiled
64-wide matmuls run concurrently (full 128x128 array, 1 MAC/cell/cycle).
"""

import sys

import numpy as np

if "/opt/trn_rl_repo" not in sys.path:
    sys.path.insert(0, "/opt/trn_rl_repo")

import ml_dtypes

N = 16384
F = 64
NCORES = 8
NB = N // NCORES  # 2048 columns (= output rows) per core
P = 128
KT = N // P  # 128 k-tiles of 128 source rows each
MM_N = 512  # moving-operand free dim per matmul (one PSUM bank of f32)
DMA_BATCH = 4  # k-tiles per dma_start
APOOL_BUFS = 6  # in-flight A-tile slots (prefetch depth)
ALT_DMA = True  # alternate A-tile DMAs between the SP and ACT HWDGE rings
SZ = 512.0  # z pre-scale for fp8 (power of 2)
SR = 16.0  # residual scale (power of 2)
MODE = "dr"  # variant kernel() uses

_BASS_CACHE: dict = {}


def _mode_flags(mode):
    fp8 = mode in ("fp8", "fp8pair", "dma8", "mm8", "mmpair")
    pair = mode in ("pair", "fp8pair", "mmpair")
    dr = mode in ("dr", "mmdr", "mmdr1", "dmadr", "dr2", "mmdr2", "dmadr2")
    dr2 = mode in ("dr2", "mmdr2", "dmadr2")
    resident = mode in ("mm", "mm8", "mmpair", "mmdr", "mmdr1", "mmdr2")
    dmaonly = mode in ("dma", "dma8", "dmadr", "dmadr2")
    return fp8, pair, dr, dr2, resident, dmaonly


def _build_bass(reps: int = 1, mode: str = "full"):
    """Build the per-core Bass module. reps>1 repeats the whole compute
    (same inputs/outputs) inside one NEFF -- used only for benchmarking
    device time independent of dispatch overhead. Probe modes: "dma*"
    (loads only, no matmul), "mm*" (matmuls from a single resident tile,
    1/64th of the DMA traffic)."""
    key = (reps, mode, DMA_BATCH, APOOL_BUFS, ALT_DMA)
    if key in _BASS_CACHE:
        return _BASS_CACHE[key]

    import concourse.mybir as mybir
    import concourse.tile as tile
    from concourse import bacc

    nc = bacc.Bacc("TRN2", target_bir_lowering=False, debug=False,
                   num_devices=NCORES)

    fp8, pair, dr, dr2, resident, dmaonly = _mode_flags(mode)
    a_dt = mybir.dt.float8e4 if (fp8 or dr) else mybir.dt.bfloat16
    # pair mode: two col-group-tiled matmuls run concurrently. Each 512-chunk
    # nn gets its own PSUM bank (columns nn*512) with even chunks on
    # partitions 0-63 and odd chunks on 64-127, so no two accumulation
    # groups share a bank.
    b_p = 2 * F if pair else F       # bias/dinv partition count
    a_in = nc.dram_tensor("a", [N, NB], a_dt, kind="ExternalInput")
    if dr:
        z_in = nc.dram_tensor("z", [P, KT * P], mybir.dt.float8e4,
                              kind="ExternalInput")
        d_in = nc.dram_tensor("dinv", [F, NB], mybir.dt.bfloat16,
                              kind="ExternalInput")
        f_in = nc.dram_tensor("fold", [P, F], mybir.dt.bfloat16,
                              kind="ExternalInput")
        b_in = nc.dram_tensor("bvec", [F, 1], mybir.dt.float32,
                              kind="ExternalInput")
        o_out = nc.dram_tensor("o", [F, NB], mybir.dt.bfloat16,
                               kind="ExternalOutput")
    else:
        z_in = nc.dram_tensor("z", [P, KT * F], mybir.dt.bfloat16,
                              kind="ExternalInput")
        b_in = nc.dram_tensor("bvec", [b_p, 1], mybir.dt.float32,
                              kind="ExternalInput")
        if fp8:
            d_in = nc.dram_tensor("dinv", [b_p, NB], mybir.dt.float32,
                                  kind="ExternalInput")
        o_out = nc.dram_tensor("o", [F, NB], mybir.dt.float32,
                               kind="ExternalOutput")

    kb = DMA_BATCH
    assert not dr or kb % 2 == 0
    # [KT/kb, 128, kb, NB]: group kb consecutive k-tiles into one DMA
    a_tiles = a_in.ap().rearrange("(g t p) i -> g p t i", t=kb, p=P)

    relu = mybir.ActivationFunctionType.Relu
    NCH = NB // MM_N  # 512-wide output chunks

    with tile.TileContext(nc) as tc:
        with (
            tc.tile_pool(name="singles", bufs=1) as singles,
            tc.tile_pool(name="apool", bufs=APOOL_BUFS) as apool,
            tc.tile_pool(name="psum", bufs=1, space="PSUM") as psum_pool,
            tc.tile_pool(name="psum2", bufs=2, space="PSUM") as psum2_pool,
        ):
            if dr2:
                # z: [128, KT tiles, 2 members (z8|r8), 64 features]
                z_sb = singles.tile([P, KT, 2, F], mybir.dt.float8e4)
                nc.sync.dma_start(
                    z_sb[:], z_in.ap().rearrange("p (kt i f) -> p kt i f",
                                                 i=2, f=F))
            elif dr:
                # z pairs: [128, KT/2 pairs, 2 members, 128 cols (z8|r8)]
                z_sb = singles.tile([P, KT // 2, 2, P], mybir.dt.float8e4)
                nc.sync.dma_start(
                    z_sb[:], z_in.ap().rearrange("p (j i m) -> p j i m",
                                                 i=2, m=P))
            else:
                z_sb = singles.tile([P, KT * F], mybir.dt.bfloat16)
                nc.sync.dma_start(z_sb[:], z_in.ap())
                if fp8:
                    d_sb = singles.tile([b_p, NB], mybir.dt.float32,
                                        tag="d_sb")
                    nc.sync.dma_start(d_sb[:], d_in.ap())
            if dr:
                d_sb = singles.tile([F, NB], mybir.dt.bfloat16, tag="d_sb")
                nc.sync.dma_start(d_sb[:], d_in.ap())
                f_sb = singles.tile([P, F], mybir.dt.bfloat16, tag="f_sb")
                nc.sync.dma_start(f_sb[:], f_in.ap())
            b_sb = singles.tile([F if dr else b_p, 1], mybir.dt.float32)
            nc.sync.dma_start(b_sb[:], b_in.ap())

            mm_tile = None
            if resident:
                mm_tile = singles.tile([P, kb, NB], a_dt, tag="mm_tile")
                nc.sync.dma_start(mm_tile[:], a_tiles[0])

            for _rep in range(reps):
                if dr:
                    ps = psum_pool.tile([P, NB], mybir.dt.float32)
                    pss = [ps[:, nn * MM_N:(nn + 1) * MM_N]
                           for nn in range(NCH)]
                    perf = mybir.MatmulPerfMode.DoubleRow
                    for g in range(KT // kb):
                        if resident:
                            at = mm_tile
                        else:
                            at = apool.tile([P, kb, NB], a_dt)
                            eng = nc.scalar if (ALT_DMA and g % 2) else nc.sync
                            eng.dma_start(at[:], a_tiles[g])
                        if dmaonly:
                            continue
                        if dr2:
                            # two concurrent 64-col DoubleRow chains: even
                            # k-tiles on col-group 0 (psum rows 0-63), odd on
                            # col-group 1 (rows 64-127). Each matmul pairs
                            # (z8, r8) against the SAME A tile (stride-0
                            # broadcast) so the residual add is free.
                            for t in range(kb):
                                kt = g * kb + t
                                h = kt % 2
                                zkt = z_sb[:, kt, :, :]
                                for nn in range(NCH):
                                    sf = slice(nn * MM_N, (nn + 1) * MM_N)
                                    nc.tensor.matmul(
                                        pss[nn][h * F:(h + 1) * F, :],
                                        lhsT=zkt,
                                        rhs=at[:, t:t + 1, sf]
                                        .broadcast_to([P, 2, MM_N]),
                                        start=(kt <= 1),
                                        stop=(kt >= KT - 2),
                                        perf_mode=perf,
                                        tile_position=(0, h * F),
                                    )
                            continue
                        for jj in range(kb // 2):
                            j = g * (kb // 2) + jj
                            zpair = z_sb[:, 0 if mode == "mmdr1"
                                         else j % (KT // 2), :, :]
                            for nn in range(NCH):
                                nc.tensor.matmul(
                                    pss[nn][:],
                                    lhsT=zpair,
                                    rhs=at[:, 2 * jj:2 * jj + 2,
                                           nn * MM_N:(nn + 1) * MM_N],
                                    start=(j == 0),
                                    stop=(j == KT // 2 - 1),
                                    perf_mode=perf,
                                )

                    s_sb = singles.tile([P, NB], mybir.dt.bfloat16,
                                        tag="s_sb")
                    o1_sb = singles.tile([F, NB], mybir.dt.bfloat16,
                                         tag="o1_sb")
                    out_sb = singles.tile([F, NB], mybir.dt.bfloat16,
                                          tag="out_sb")
                    if dmaonly:
                        nc.vector.tensor_copy(out_sb[:, :F],
                                              z_sb[:F, 0, 0, :F])
                        nc.sync.dma_start(o_out.ap(), out_sb[:])
                    else:
                        for nn in range(NCH):
                            sf = slice(nn * MM_N, (nn + 1) * MM_N)
                            nc.vector.tensor_copy(s_sb[:, sf], pss[nn][:])
                            ps2 = psum2_pool.tile([F, MM_N], mybir.dt.float32)
                            nc.tensor.matmul(ps2[:], lhsT=f_sb[:],
                                             rhs=s_sb[:, sf],
                                             start=True, stop=True)
                            nc.vector.tensor_mul(o1_sb[:, sf], ps2[:],
                                                 d_sb[:, sf])
                            nc.scalar.activation(out_sb[:, sf], o1_sb[:, sf],
                                                 relu, bias=b_sb[:],
                                                 scale=1.0)
                            nc.sync.dma_start(o_out.ap()[:, sf],
                                              out_sb[:, sf])
                    continue

                ps = psum_pool.tile([b_p, NB], mybir.dt.float32)

                for g in range(KT // kb):
                    if resident:
                        at = mm_tile
                    else:
                        at = apool.tile([P, kb, NB], a_dt)
                        eng = nc.scalar if (ALT_DMA and g % 2) else nc.sync
                        eng.dma_start(at[:], a_tiles[g])
                    if dmaonly:
                        continue
                    for t in range(kb):
                        kt = g * kb + t
                        zk = z_sb[:, (kt % KT) * F:((kt % KT) + 1) * F]
                        if pair:
                            for nn in range(NCH):
                                h = nn % 2
                                nc.tensor.matmul(
                                    ps[h * F:(h + 1) * F,
                                       nn * MM_N:(nn + 1) * MM_N],
                                    lhsT=zk,
                                    rhs=at[:, t, nn * MM_N:(nn + 1) * MM_N],
                                    start=(kt == 0),
                                    stop=(kt == KT - 1),
                                    tile_position=(0, h * F),
                                )
                        else:
                            for nn in range(NCH):
                                nc.tensor.matmul(
                                    ps[:, nn * MM_N:(nn + 1) * MM_N],
                                    lhsT=zk,
                                    rhs=at[:, t, nn * MM_N:(nn + 1) * MM_N],
                                    start=(kt == 0),
                                    stop=(kt == KT - 1),
                                )

                out_sb = singles.tile([b_p, NB], mybir.dt.float32,
                                      tag="out_sb")
                if dmaonly:
                    nc.vector.tensor_copy(out_sb[:F, :F], z_sb[:F, :F])
                    nc.sync.dma_start(o_out.ap(), out_sb[:F, :])
                elif pair:
                    # touch only the written PSUM quadrants
                    for nn in range(NCH):
                        h = nn % 2
                        sp = slice(h * F, (h + 1) * F)
                        sf = slice(nn * MM_N, (nn + 1) * MM_N)
                        if fp8:
                            nc.vector.tensor_mul(out_sb[sp, sf], ps[sp, sf],
                                                 d_sb[sp, sf])
                            nc.scalar.activation(out_sb[sp, sf],
                                                 out_sb[sp, sf], relu,
                                                 bias=b_sb[sp], scale=1.0)
                        else:
                            nc.scalar.activation(out_sb[sp, sf], ps[sp, sf],
                                                 relu, bias=b_sb[sp],
                                                 scale=1.0)
                        nc.sync.dma_start(o_out.ap()[:, sf], out_sb[sp, sf])
                elif fp8:
                    nc.vector.tensor_mul(out_sb[:], ps[:], d_sb[:])
                    nc.scalar.activation(out_sb[:], out_sb[:], relu,
                                         bias=b_sb[:], scale=1.0)
                    nc.sync.dma_start(o_out.ap(), out_sb[:])
                else:
                    nc.scalar.activation(out_sb[:], ps[:], relu,
                                         bias=b_sb[:], scale=1.0)
                    nc.sync.dma_start(o_out.ap(), out_sb[:])

    nc.compile()
    _BASS_CACHE[key] = nc
    return nc


def _host_prep(x, adj, W, b, mode=None):
    """Host-side sharding/preprocessing -> per-core input maps."""
    if mode is None:
        mode = MODE
    fp8, pair, dr, dr2, resident, dmaonly = _mode_flags(mode)
    x = np.asarray(x, dtype=np.float32)
    adj = np.asarray(adj, dtype=np.float32)
    W = np.asarray(W, dtype=np.float32)
    b = np.asarray(b, dtype=np.float32)

    deg = adj.sum(axis=0) + 1.0
    dinv = np.where(deg > 0, 1.0 / np.sqrt(deg), 0.0).astype(np.float32)

    z = (dinv[:, None] * (x @ W)).astype(np.float32)  # [N, F]

    idx = np.arange(NB)
    in_maps = []

    if dr:
        zq = z * SZ
        z8 = zq.astype(ml_dtypes.float8_e4m3)
        fold = np.zeros((P, F), np.float32)
        fold[np.arange(F), np.arange(F)] = 1.0 / SZ
        if dr2:
            r = zq - z8.astype(np.float32)
            r8 = r.astype(ml_dtypes.float8_e4m3)  # unscaled residual
            # z_dev[p, kt*128 + i*64 + f] = (z8 if i==0 else r8)[kt*128+p, f]
            zr = np.stack([z8.reshape(KT, P, F), r8.reshape(KT, P, F)],
                          axis=2)  # [KT, P, 2, F]
            z_dev = np.ascontiguousarray(
                zr.transpose(1, 0, 2, 3).reshape(P, KT * P))
            fold[F + np.arange(F), np.arange(F)] = 1.0 / SZ
        else:
            r = (zq - z8.astype(np.float32)) * SR
            r8 = r.astype(ml_dtypes.float8_e4m3)
            zr = np.concatenate([z8, r8], axis=1)  # [N, 128] fp8
            # z_dev[p, j*256 + i*128 + m] = zr[(2j+i)*128 + p, m]
            z_dev = np.ascontiguousarray(
                zr.reshape(KT // 2, 2, P, P).transpose(2, 0, 1, 3)
                .reshape(P, KT * P))
            fold[F + np.arange(F), np.arange(F)] = 1.0 / (SZ * SR)
        fold = fold.astype(ml_dtypes.bfloat16)
        b_dev = np.ascontiguousarray(b.reshape(F, 1))
        for c in range(NCORES):
            cs = c * NB
            blk = adj[:, cs:cs + NB].copy()
            blk[cs + idx, idx] += 1.0  # self-loop (+I)
            d64 = np.ascontiguousarray(
                np.broadcast_to(dinv[cs:cs + NB], (F, NB))
            ).astype(ml_dtypes.bfloat16)
            in_maps.append({
                "a": blk.astype(ml_dtypes.float8_e4m3),
                "z": z_dev,
                "dinv": d64,
                "fold": fold,
                "bvec": b_dev,
            })
        return in_maps

    # k-major layout: z_sb[p, kt*F + f] = z[kt*128 + p, f]
    z_dev = np.ascontiguousarray(
        z.reshape(KT, P, F).transpose(1, 0, 2).reshape(P, KT * F)
    ).astype(ml_dtypes.bfloat16)

    if pair:
        b_dev = np.ascontiguousarray(
            np.concatenate([b, b]).reshape(2 * F, 1))
    else:
        b_dev = np.ascontiguousarray(b.reshape(F, 1))

    def _pair_dinv(dc):
        # [128, NB]: chunk nn lives at [64*(nn%2):64*(nn%2+1), nn*512:...]
        d = np.zeros((2 * F, NB), np.float32)
        for nn in range(NB // MM_N):
            h = nn % 2
            d[h * F:(h + 1) * F, nn * MM_N:(nn + 1) * MM_N] = \
                dc[nn * MM_N:(nn + 1) * MM_N]
        return d

    for c in range(NCORES):
        cs = c * NB
        if fp8:
            # adjacency stays exact {0,1,2} in fp8; dinv applied on device
            blk = adj[:, cs:cs + NB].copy()
            blk[cs + idx, idx] += 1.0  # self-loop (+I)
            dc = dinv[cs:cs + NB]
            m = {
                "a": blk.astype(ml_dtypes.float8_e4m3),
                "z": z_dev,
                "bvec": b_dev,
                "dinv": (_pair_dinv(dc) if pair else np.ascontiguousarray(
                    np.broadcast_to(dc, (F, NB)))),
            }
        else:
            blk = adj[:, cs:cs + NB] * dinv[cs:cs + NB][None, :]
            blk[cs + idx, idx] += dinv[cs + idx]  # fold self-loop (+I)
            m = {
                "a": blk.astype(ml_dtypes.bfloat16),
                "z": z_dev,
                "bvec": b_dev,
            }
        in_maps.append(m)
    return in_maps


def _assemble(results, mode=None):
    """Device outputs -> full [N, F] output."""
    if mode is None:
        mode = MODE
    out = np.empty((N, F), dtype=np.float32)
    for c in range(NCORES):
        o = results[c]["o"]
        out[c * NB:(c + 1) * NB, :] = o.astype(np.float32).T
    return out


def kernel(x, adj, W, b):
    from concourse import bass_utils

    nc = _build_bass(mode=MODE)
    in_maps = _host_prep(x, adj, W, b, mode=MODE)
    res = bass_utils.run_bass_kernel_spmd(nc, in_maps,
                                          core_ids=list(range(NCORES)))
    return _assemble(res.results, mode=MODE)
